# revision 38
# baseline (speedup 1.0000x reference)
"""MoE (dense routing) Trainium2 kernel.

Data-parallel over the batch: each of the 8 NeuronCores processes
B/8 = 2048 rows with the full set of gate + expert weights.

Per-core dataflow (feature-major activations, i.e. [features, batch]):
  gates:  g1 = relu(gw1.T @ xT);  g2 = relu(gw2.T @ g1)
          z_e = gw3[:, e].T @ g2      (M=1 matmuls -> partition 0)
          E_e = exp(z_e)              (unnormalized)
  expert: h1 = relu(ew1[e].T @ xT); h2 = relu(ew2[e].T @ h1)
          h2s = h2 * broadcast(E_e)   (gate folded in before layer 3)
          acc += ew3[e].T @ h2s [+ eb3[e] outer E_e]
  out = acc * broadcast(1 / sum_e E_e)   (softmax denominator at the end)
All matmuls run in float32r (full fp32 data, 1 cycle/row on the PE).
"""

import os
import sys
import types
import numpy as np
from contextlib import ExitStack

import concourse.bass as bass
import concourse.tile as tile
from concourse import bacc, mybir
from concourse import bass_utils

# bass_utils imports antenv.axon_hooks when tracing is requested (e.g. a
# stray BASS_TRACE in the environment); provide a no-op stub if the
# image's antenv lacks it so the run degrades to untraced instead of
# crashing.
try:
    import antenv.axon_hooks  # noqa: F401
except ImportError:
    _stub = types.ModuleType("antenv.axon_hooks")
    _stub.get_axon_ntff_profile_hook = lambda: None
    _stub.set_axon_ntff_profile_hook = lambda hook: None
    sys.modules["antenv.axon_hooks"] = _stub

F32 = mybir.dt.float32
F32R = mybir.dt.float32r
AF = mybir.ActivationFunctionType

B, D, H, O, E = 16384, 1024, 512, 256, 8
NCORES = 8
BL = B // NCORES          # 2048 rows per core
BS = 512                  # moving-operand chunk (fp32 max = 512)
NBS = BL // BS            # 4
G1, G2 = 256, 128         # gate hidden sizes
KD = D // 128             # 8 contraction subtiles for D
KH = H // 128             # 4 for H

_CACHE = {}


def _build(with_eb3):
    key = ("nc", with_eb3)
    if key in _CACHE:
        return _CACHE[key]

    nc = bacc.Bacc("TRN2", target_bir_lowering=False, debug=False)

    xT4 = nc.dram_tensor("xT4", [NBS, 128, KD, BS], F32R, kind="ExternalInput").ap()
    gw1 = nc.dram_tensor("gw1", [128, KD, G1], F32R, kind="ExternalInput").ap()
    gw2 = nc.dram_tensor("gw2", [128, G1 // 128, G2], F32R, kind="ExternalInput").ap()
    gw3 = nc.dram_tensor("gw3", [G2, E], F32R, kind="ExternalInput").ap()
    gb1t = nc.dram_tensor("gb1t", [128, 2], F32, kind="ExternalInput").ap()
    gb2t = nc.dram_tensor("gb2t", [128, 1], F32, kind="ExternalInput").ap()
    gb3t = nc.dram_tensor("gb3t", [E, 1], F32, kind="ExternalInput").ap()
    ones8d = nc.dram_tensor("ones8d", [E, 1], F32R, kind="ExternalInput").ap()
    ew1 = nc.dram_tensor("ew1", [E, 128, KD, H], F32R, kind="ExternalInput").ap()
    ew2 = nc.dram_tensor("ew2", [E, 128, KH, H], F32R, kind="ExternalInput").ap()
    ew3 = nc.dram_tensor("ew3", [E, 128, KH, O], F32R, kind="ExternalInput").ap()
    eb1t = nc.dram_tensor("eb1t", [128, E, 4], F32, kind="ExternalInput").ap()
    eb2t = nc.dram_tensor("eb2t", [128, E, 4], F32, kind="ExternalInput").ap()
    eb3f = nc.dram_tensor("eb3f", [1, E * O], F32R, kind="ExternalInput").ap()
    outT = nc.dram_tensor("outT", [O, BL], F32, kind="ExternalOutput").ap()

    with tile.TileContext(nc) as tc, ExitStack() as ctx:
        pers = ctx.enter_context(tc.tile_pool(name="pers", bufs=1))
        dramp = ctx.enter_context(tc.tile_pool(name="dramp", bufs=1, space="DRAM"))
        gates_dram = dramp.tile([E, BL], F32R)

        # gate weights first on the fast ring (small, unblock matmul #0)
        gw1s = pers.tile([128, KD, G1], F32R)
        nc.gpsimd.dma_start(gw1s[:].rearrange("p k b -> p (k b)"), gw1.rearrange("p k b -> p (k b)"))
        gw2s = pers.tile([128, G1 // 128, G2], F32R)
        nc.gpsimd.dma_start(gw2s[:], gw2)
        gw3s = pers.tile([128, E], F32R)
        nc.gpsimd.dma_start(gw3s[:], gw3)
        gb1s = pers.tile([128, 2], F32)
        nc.sync.dma_start(gb1s[:], gb1t)
        gb2s = pers.tile([128, 1], F32)
        nc.sync.dma_start(gb2s[:], gb2t)
        gb3s = pers.tile([E, 1], F32)
        nc.sync.dma_start(gb3s[:], gb3t)
        ones8 = pers.tile([E, 1], F32R)
        nc.sync.dma_start(ones8[:], ones8d)
        eb1s = pers.tile([128, E, 4], F32)
        nc.sync.dma_start(eb1s[:], eb1t)
        eb2s = pers.tile([128, E, 4], F32)
        nc.sync.dma_start(eb2s[:], eb2t)
        if with_eb3:
            eb3s = pers.tile([1, E * O], F32R)
            nc.sync.dma_start(eb3s[:], eb3f)

        # x, feature-major, one tile per batch chunk (host-rearranged so
        # every DMA descriptor is 16KB contiguous per partition).  The
        # gpsimd SWDGE ring sustains ~240 GB/s vs ~50-100 GB/s for the
        # HWDGE rings, so everything big streams there, ordered by need.
        xtb = [pers.tile([128, KD, BS], F32R, tag=f"xtb{i}", name=f"xtb{i}") for i in range(NBS)]
        nc.gpsimd.dma_start(xtb[0][:].rearrange("p k b -> p (k b)"), xT4[0].rearrange("p k b -> p (k b)"))

        outacc = pers.tile([128, 2, BL], F32)
        rbc = pers.tile([128, NBS, BS], F32)   # 1/sum_e exp broadcast tiles

        # expert weight streaming on the gpsimd queue (doesn't contend
        # with the gate-phase sync/scalar traffic)
        with tc.tile_pool(name="wp", bufs=2) as wp:
            def load_expert(e):
                w1 = wp.tile([128, KD, H], F32R, tag="w1")
                nc.gpsimd.dma_start(w1[:].rearrange("p k b -> p (k b)"), ew1[e].rearrange("p k b -> p (k b)"))
                w2 = wp.tile([128, KH, H], F32R, tag="w2")
                nc.gpsimd.dma_start(w2[:].rearrange("p k b -> p (k b)"), ew2[e].rearrange("p k b -> p (k b)"))
                w3 = wp.tile([128, KH, O], F32R, tag="w3")
                nc.gpsimd.dma_start(w3[:].rearrange("p k b -> p (k b)"), ew3[e].rearrange("p k b -> p (k b)"))
                return w1, w2, w3

            # interleave expert-0 weights with the remaining x chunks in
            # the order the PE will need them
            w1_0 = wp.tile([128, KD, H], F32R, tag="w1")
            nc.gpsimd.dma_start(w1_0[:].rearrange("p k b -> p (k b)"), ew1[0].rearrange("p k b -> p (k b)"))
            nc.gpsimd.dma_start(xtb[1][:].rearrange("p k b -> p (k b)"), xT4[1].rearrange("p k b -> p (k b)"))
            w2_0 = wp.tile([128, KH, H], F32R, tag="w2")
            nc.gpsimd.dma_start(w2_0[:].rearrange("p k b -> p (k b)"), ew2[0].rearrange("p k b -> p (k b)"))
            w3_0 = wp.tile([128, KH, O], F32R, tag="w3")
            nc.gpsimd.dma_start(w3_0[:].rearrange("p k b -> p (k b)"), ew3[0].rearrange("p k b -> p (k b)"))
            nc.scalar.dma_start(xtb[2][:].rearrange("p k b -> p (k b)"), xT4[2].rearrange("p k b -> p (k b)"))
            nc.sync.dma_start(xtb[3][:].rearrange("p k b -> p (k b)"), xT4[3].rearrange("p k b -> p (k b)"))

            # ---------------- gate phase ----------------
            # Software-pipelined emission (G1 groups run ahead of G2
            # ahead of the z/softmax stage) so the PE never waits on the
            # relu chain between batch chunks.
            rlist = []
            with tc.tile_pool(name="gtmp", bufs=3) as gtmp, \
                 tc.tile_pool(name="gps", bufs=4, space="PSUM") as gps, \
                 tc.tile_pool(name="zps", bufs=2, space="PSUM") as zps, \
                 tc.tile_pool(name="z2ps", bufs=1, space="PSUM") as z2ps:
                g1sd, g2sd = {}, {}

                def stage_g1(bs):
                    g1s = gtmp.tile([128, 2, BS], F32R, tag="g1s", name="g1s")
                    for m in range(2):
                        p = gps.tile([128, BS], F32, tag="g1p")
                        for k in range(KD):
                            nc.tensor.matmul(p[:], gw1s[:, k, m * 128:(m + 1) * 128],
                                             xtb[bs][:, k, :],
                                             start=(k == 0), stop=(k == KD - 1))
                        nc.scalar.activation(g1s[:, m], p[:], AF.Relu,
                                             bias=gb1s[:, m:m + 1])
                    g1sd[bs] = g1s

                def stage_g2(bs):
                    g2s = gtmp.tile([128, BS], F32R, tag="g2s", name="g2s")
                    p = zps.tile([128, BS], F32, tag="g2p")
                    for k in range(2):
                        nc.tensor.matmul(p[:], gw2s[:, k, :], g1sd[bs][:, k, :],
                                         start=(k == 0), stop=(k == 1))
                    nc.scalar.activation(g2s[:], p[:], AF.Relu, bias=gb2s[:, 0:1])
                    g2sd[bs] = g2s

                def stage_z(bs):
                    bsl = slice(bs * BS, (bs + 1) * BS)
                    zp8 = z2ps.tile([E, BS], F32, tag="zp8", name="zp8")
                    nc.tensor.matmul(zp8[:], gw3s[:, :E], g2sd[bs][:],
                                     start=True, stop=True)
                    E8 = gtmp.tile([E, BS], F32R, tag="E8", name="E8")
                    nc.scalar.activation(E8[:], zp8[:], AF.Exp, bias=gb3s[:])
                    # unnormalized gates to DRAM for the expert loop
                    nc.scalar.dma_start(gates_dram[:, bsl], E8[:])
                    # softmax denominator via ones-matmul -> partition 0
                    sp = z2ps.tile([1, BS], F32, tag="sp", name="sp")
                    nc.tensor.matmul(sp[:], ones8[:], E8[:], start=True, stop=True)
                    # bounce to SBUF so the PSUM bank frees immediately
                    # (the reciprocal queues behind earlier DVE work)
                    spb = gtmp.tile([1, BS], F32, tag="spb", name="spb")
                    nc.scalar.activation(spb[:], sp[:], AF.Copy)
                    R = pers.tile([1, BS], F32, tag=f"R{bs}", name="R")
                    nc.vector.reciprocal(R[:], spb[:])
                    rlist.append(R)

                stage_g1(0)
                stage_g1(1)
                stage_g2(0)
                stage_g1(2)
                stage_g2(1)
                stage_z(0)
                stage_g1(3)
                stage_g2(2)
                stage_z(1)
                stage_g2(3)
                stage_z(2)
                stage_z(3)

            # ---------------- expert phase ----------------
            outTr = outT.rearrange("(mo p) b -> p mo b", p=128)
            with tc.tile_pool(name="hp", bufs=2) as hp, \
                 tc.tile_pool(name="bp", bufs=3) as bp, \
                 tc.tile_pool(name="eps", bufs=3, space="PSUM") as eps, \
                 tc.tile_pool(name="ops", bufs=2, space="PSUM") as ops:
                for e in range(E):
                    if e == 0:
                        w1, w2, w3 = w1_0, w2_0, w3_0
                    else:
                        w1, w2, w3 = load_expert(e)
                    if e == 2:
                        # denominator broadcasts, queued here so they
                        # neither delay the early expert weight DMAs nor
                        # the final scale at the tail
                        for bs2 in range(NBS):
                            nc.gpsimd.partition_broadcast(rbc[:, bs2, :],
                                                          rlist[bs2][:])
                    for bs in range(NBS):
                        bsl = slice(bs * BS, (bs + 1) * BS)
                        ge = bp.tile([1, BS], F32R, tag="ge")
                        nc.sync.dma_start(ge[:], gates_dram[e:e + 1, bsl])
                        gbc = bp.tile([128, BS], F32, tag="gbc")
                        nc.gpsimd.partition_broadcast(gbc[:].bitcast(F32R), ge[:])

                        h1s = hp.tile([128, KH, BS], F32R, tag="h")
                        for m in range(KH):
                            p = eps.tile([128, BS], F32, tag="h1p")
                            for k in range(KD):
                                nc.tensor.matmul(p[:], w1[:, k, m * 128:(m + 1) * 128],
                                                 xtb[bs][:, k, :],
                                                 start=(k == 0), stop=(k == KD - 1))
                            nc.scalar.activation(h1s[:, m], p[:], AF.Relu,
                                                 bias=eb1s[:, e, m:m + 1])
                        h2s = hp.tile([128, KH, BS], F32R, tag="h")
                        for m in range(KH):
                            p = eps.tile([128, BS], F32, tag="h2p")
                            for k in range(KH):
                                nc.tensor.matmul(p[:], w2[:, k, m * 128:(m + 1) * 128],
                                                 h1s[:, k, :],
                                                 start=(k == 0), stop=(k == KH - 1))
                            t2 = bp.tile([128, BS], F32, tag="t2")
                            nc.scalar.activation(t2[:], p[:], AF.Relu,
                                                 bias=eb2s[:, e, m:m + 1])
                            nc.vector.tensor_mul(h2s[:, m], t2[:], gbc[:])
                        for mo in range(O // 128):
                            p = ops.tile([128, BS], F32, tag="op")
                            nmm = KH + (1 if with_eb3 else 0)
                            for k in range(KH):
                                nc.tensor.matmul(p[:], w3[:, k, mo * 128:(mo + 1) * 128],
                                                 h2s[:, k, :],
                                                 start=(k == 0), stop=(k == nmm - 1))
                            if with_eb3:
                                nc.tensor.matmul(p[:], eb3s[:, e * O + mo * 128:
                                                            e * O + (mo + 1) * 128],
                                                 ge[:], start=False, stop=True)
                            if e == 0:
                                nc.vector.tensor_copy(outacc[:, mo, bsl], p[:])
                            elif e < E - 1:
                                nc.vector.tensor_add(outacc[:, mo, bsl],
                                                     outacc[:, mo, bsl], p[:])
                            else:
                                # last expert: fold in the softmax
                                # denominator and stream the chunk out
                                nc.vector.scalar_tensor_tensor(
                                    outacc[:, mo, bsl], p[:], 1.0,
                                    outacc[:, mo, bsl],
                                    mybir.AluOpType.mult,
                                    mybir.AluOpType.add)
                                nc.vector.tensor_mul(outacc[:, mo, bsl],
                                                     outacc[:, mo, bsl],
                                                     rbc[:, bs, :])
                        if e == E - 1:
                            nc.gpsimd.dma_start(outTr[:, :, bsl],
                                                outacc[:, :, bsl])

    nc.compile()
    _CACHE[key] = nc
    return nc


def kernel(x, gw1, gb1, gw2, gb2, gw3, gb3, ew1, eb1, ew2, eb2, ew3, eb3):
    x = np.asarray(x, dtype=np.float32)
    # [D_in, D_out] -> [128, KD, D_out] partition-major (16KB descriptors)
    pm = lambda w, kd: np.ascontiguousarray(
        np.asarray(w, np.float32).reshape(kd, 128, -1).transpose(1, 0, 2))
    gw1 = pm(gw1, KD)
    gw2 = pm(gw2, G1 // 128)
    gw3 = np.ascontiguousarray(np.asarray(gw3, dtype=np.float32))
    ew1 = np.ascontiguousarray(np.asarray(ew1, np.float32)
                               .reshape(E, KD, 128, H).transpose(0, 2, 1, 3))
    ew2 = np.ascontiguousarray(np.asarray(ew2, np.float32)
                               .reshape(E, KH, 128, H).transpose(0, 2, 1, 3))
    ew3 = np.ascontiguousarray(np.asarray(ew3, np.float32)
                               .reshape(E, KH, 128, O).transpose(0, 2, 1, 3))
    gb1t = np.ascontiguousarray(np.asarray(gb1, np.float32).reshape(2, 128).T)
    gb2t = np.ascontiguousarray(np.asarray(gb2, np.float32).reshape(1, 128).T)
    gb3t = np.ascontiguousarray(np.asarray(gb3, np.float32).reshape(E, 1))
    eb1t = np.ascontiguousarray(
        np.asarray(eb1, np.float32).reshape(E, 4, 128).transpose(2, 0, 1))
    eb2t = np.ascontiguousarray(
        np.asarray(eb2, np.float32).reshape(E, 4, 128).transpose(2, 0, 1))
    eb3f = np.ascontiguousarray(np.asarray(eb3, np.float32).reshape(1, E * O))

    with_eb3 = bool(np.any(eb3f))
    nc = _build(with_eb3)

    shared = {
        "gw1": gw1, "gw2": gw2, "gw3": gw3,
        "gb1t": gb1t, "gb2t": gb2t, "gb3t": gb3t,
        "ones8d": np.ones((E, 1), np.float32),
        "ew1": ew1, "ew2": ew2, "ew3": ew3,
        "eb1t": eb1t, "eb2t": eb2t, "eb3f": eb3f,
    }
    in_maps = []
    for c in range(NCORES):
        # [BL, D] -> [NBS, 128, KD, BS]: xT4[bs, p, ko, b] = x[bs*BS+b, ko*128+p]
        xc = x[c * BL:(c + 1) * BL, :].reshape(NBS, BS, KD, 128)
        xTc = np.ascontiguousarray(xc.transpose(0, 3, 2, 1))
        in_maps.append({"xT4": xTc, **shared})

    trace = os.environ.get("MOE_TRACE", "0") == "1"
    res = bass_utils.run_bass_kernel_spmd(
        nc, in_maps, core_ids=list(range(NCORES)), trace=trace)
    if trace:
        _CACHE["last_exec_time_ns"] = res.exec_time_ns
        _CACHE["last_results"] = res

    out = np.empty((B, O), dtype=np.float32)
    for c in range(NCORES):
        out[c * BL:(c + 1) * BL, :] = res.results[c]["outT"].T
    return out


# revision 40
# speedup vs baseline: 1.0291x; 1.0291x over previous
"""MoE (dense routing) Trainium2 kernel.

Data-parallel over the batch: each of the 8 NeuronCores processes
B/8 = 2048 rows with the full set of gate + expert weights.

Per-core dataflow (feature-major activations, i.e. [features, batch]):
  gates:  g1 = relu(gw1.T @ xT);  g2 = relu(gw2.T @ g1)
          z_e = gw3[:, e].T @ g2      (M=1 matmuls -> partition 0)
          E_e = exp(z_e)              (unnormalized)
  expert: h1 = relu(ew1[e].T @ xT); h2 = relu(ew2[e].T @ h1)
          h2s = h2 * broadcast(E_e)   (gate folded in before layer 3)
          acc += ew3[e].T @ h2s [+ eb3[e] outer E_e]
  out = acc * broadcast(1 / sum_e E_e)   (softmax denominator at the end)
All matmuls run in float32r (full fp32 data, 1 cycle/row on the PE).
"""

import os
import sys
import types
import numpy as np
from contextlib import ExitStack

import concourse.bass as bass
import concourse.tile as tile
from concourse import bacc, mybir
from concourse import bass_utils

# bass_utils imports antenv.axon_hooks when tracing is requested (e.g. a
# stray BASS_TRACE in the environment); provide a no-op stub if the
# image's antenv lacks it so the run degrades to untraced instead of
# crashing.
try:
    import antenv.axon_hooks  # noqa: F401
except ImportError:
    _stub = types.ModuleType("antenv.axon_hooks")
    _stub.get_axon_ntff_profile_hook = lambda: None
    _stub.set_axon_ntff_profile_hook = lambda hook: None
    sys.modules["antenv.axon_hooks"] = _stub

F32 = mybir.dt.float32
F32R = mybir.dt.float32r
AF = mybir.ActivationFunctionType

B, D, H, O, E = 16384, 1024, 512, 256, 8
NCORES = 8
BL = B // NCORES          # 2048 rows per core
BS = 512                  # moving-operand chunk (fp32 max = 512)
NBS = BL // BS            # 4
G1, G2 = 256, 128         # gate hidden sizes
KD = D // 128             # 8 contraction subtiles for D
KH = H // 128             # 4 for H

_CACHE = {}


def _build(with_eb3):
    key = ("nc", with_eb3)
    if key in _CACHE:
        return _CACHE[key]

    nc = bacc.Bacc("TRN2", target_bir_lowering=False, debug=False)

    xT4 = nc.dram_tensor("xT4", [NBS, 128, KD, BS], F32R, kind="ExternalInput").ap()
    gw1 = nc.dram_tensor("gw1", [128, KD, G1], F32R, kind="ExternalInput").ap()
    gw2 = nc.dram_tensor("gw2", [128, G1 // 128, G2], F32R, kind="ExternalInput").ap()
    gw3 = nc.dram_tensor("gw3", [G2, E], F32R, kind="ExternalInput").ap()
    gb1t = nc.dram_tensor("gb1t", [128, 2], F32, kind="ExternalInput").ap()
    gb2t = nc.dram_tensor("gb2t", [128, 1], F32, kind="ExternalInput").ap()
    gb3t = nc.dram_tensor("gb3t", [E, 1], F32, kind="ExternalInput").ap()
    ones8d = nc.dram_tensor("ones8d", [E, 1], F32R, kind="ExternalInput").ap()
    ew1 = nc.dram_tensor("ew1", [E, 128, KD, H], F32R, kind="ExternalInput").ap()
    ew2 = nc.dram_tensor("ew2", [E, 128, KH, H], F32R, kind="ExternalInput").ap()
    ew3 = nc.dram_tensor("ew3", [E, 128, KH, O], F32R, kind="ExternalInput").ap()
    eb1t = nc.dram_tensor("eb1t", [128, E, 4], F32, kind="ExternalInput").ap()
    eb2t = nc.dram_tensor("eb2t", [128, E, 4], F32, kind="ExternalInput").ap()
    eb3f = nc.dram_tensor("eb3f", [1, E * O], F32R, kind="ExternalInput").ap()
    outT = nc.dram_tensor("outT", [O, BL], F32, kind="ExternalOutput").ap()

    with tile.TileContext(nc) as tc, ExitStack() as ctx:
        pers = ctx.enter_context(tc.tile_pool(name="pers", bufs=1))
        dramp = ctx.enter_context(tc.tile_pool(name="dramp", bufs=1, space="DRAM"))
        gates_dram = dramp.tile([E, BL], F32R)

        # gate weights first on the fast ring (small, unblock matmul #0)
        gw1s = pers.tile([128, KD, G1], F32R)
        nc.gpsimd.dma_start(gw1s[:].rearrange("p k b -> p (k b)"), gw1.rearrange("p k b -> p (k b)"))
        gw2s = pers.tile([128, G1 // 128, G2], F32R)
        nc.gpsimd.dma_start(gw2s[:].rearrange("p k b -> p (k b)"), gw2.rearrange("p k b -> p (k b)"))
        gw3s = pers.tile([128, E], F32R)
        nc.gpsimd.dma_start(gw3s[:], gw3)
        gb1s = pers.tile([128, 2], F32)
        nc.sync.dma_start(gb1s[:], gb1t)
        gb2s = pers.tile([128, 1], F32)
        nc.sync.dma_start(gb2s[:], gb2t)
        gb3s = pers.tile([E, 1], F32)
        nc.sync.dma_start(gb3s[:], gb3t)
        ones8 = pers.tile([E, 1], F32R)
        nc.sync.dma_start(ones8[:], ones8d)
        eb1s = pers.tile([128, E, 4], F32)
        nc.sync.dma_start(eb1s[:], eb1t)
        eb2s = pers.tile([128, E, 4], F32)
        nc.sync.dma_start(eb2s[:], eb2t)
        if with_eb3:
            eb3s = pers.tile([1, E * O], F32R)
            nc.sync.dma_start(eb3s[:], eb3f)

        # x, feature-major, one tile per batch chunk (host-rearranged so
        # every DMA descriptor is 16KB contiguous per partition).  The
        # gpsimd SWDGE ring sustains ~240 GB/s vs ~50-100 GB/s for the
        # HWDGE rings, so everything big streams there, ordered by need.
        xtb = [pers.tile([128, KD, BS], F32R, tag=f"xtb{i}", name=f"xtb{i}") for i in range(NBS)]
        nc.gpsimd.dma_start(xtb[0][:].rearrange("p k b -> p (k b)"), xT4[0].rearrange("p k b -> p (k b)"))

        outacc = pers.tile([128, 2, BL], F32)
        rbc = pers.tile([128, NBS, BS], F32)   # 1/sum_e exp broadcast tiles

        # expert weight streaming on the gpsimd queue (doesn't contend
        # with the gate-phase sync/scalar traffic)
        with tc.tile_pool(name="wp", bufs=2) as wp:
            def load_expert(e):
                w1 = wp.tile([128, KD, H], F32R, tag="w1")
                nc.gpsimd.dma_start(w1[:].rearrange("p k b -> p (k b)"), ew1[e].rearrange("p k b -> p (k b)"))
                w2 = wp.tile([128, KH, H], F32R, tag="w2")
                nc.gpsimd.dma_start(w2[:].rearrange("p k b -> p (k b)"), ew2[e].rearrange("p k b -> p (k b)"))
                w3 = wp.tile([128, KH, O], F32R, tag="w3")
                nc.gpsimd.dma_start(w3[:].rearrange("p k b -> p (k b)"), ew3[e].rearrange("p k b -> p (k b)"))
                return w1, w2, w3

            # interleave expert-0 weights with the remaining x chunks in
            # the order the PE will need them
            w1_0 = wp.tile([128, KD, H], F32R, tag="w1")
            nc.gpsimd.dma_start(w1_0[:].rearrange("p k b -> p (k b)"), ew1[0].rearrange("p k b -> p (k b)"))
            nc.gpsimd.dma_start(xtb[1][:].rearrange("p k b -> p (k b)"), xT4[1].rearrange("p k b -> p (k b)"))
            w2_0 = wp.tile([128, KH, H], F32R, tag="w2")
            nc.gpsimd.dma_start(w2_0[:].rearrange("p k b -> p (k b)"), ew2[0].rearrange("p k b -> p (k b)"))
            w3_0 = wp.tile([128, KH, O], F32R, tag="w3")
            nc.gpsimd.dma_start(w3_0[:].rearrange("p k b -> p (k b)"), ew3[0].rearrange("p k b -> p (k b)"))
            nc.gpsimd.dma_start(xtb[2][:].rearrange("p k b -> p (k b)"), xT4[2].rearrange("p k b -> p (k b)"))
            nc.gpsimd.dma_start(xtb[3][:].rearrange("p k b -> p (k b)"), xT4[3].rearrange("p k b -> p (k b)"))

            # ---------------- gate phase ----------------
            # Software-pipelined emission (G1 groups run ahead of G2
            # ahead of the z/softmax stage) so the PE never waits on the
            # relu chain between batch chunks.
            rlist = []
            with tc.tile_pool(name="gtmp", bufs=3) as gtmp, \
                 tc.tile_pool(name="gps", bufs=4, space="PSUM") as gps, \
                 tc.tile_pool(name="zps", bufs=2, space="PSUM") as zps, \
                 tc.tile_pool(name="z2ps", bufs=1, space="PSUM") as z2ps:
                g1sd, g2sd = {}, {}

                def stage_g1(bs):
                    g1s = gtmp.tile([128, 2, BS], F32R, tag="g1s", name="g1s")
                    for m in range(2):
                        p = gps.tile([128, BS], F32, tag="g1p")
                        for k in range(KD):
                            nc.tensor.matmul(p[:], gw1s[:, k, m * 128:(m + 1) * 128],
                                             xtb[bs][:, k, :],
                                             start=(k == 0), stop=(k == KD - 1))
                        nc.scalar.activation(g1s[:, m], p[:], AF.Relu,
                                             bias=gb1s[:, m:m + 1])
                    g1sd[bs] = g1s

                def stage_g2(bs):
                    g2s = gtmp.tile([128, BS], F32R, tag="g2s", name="g2s")
                    p = zps.tile([128, BS], F32, tag="g2p")
                    for k in range(2):
                        nc.tensor.matmul(p[:], gw2s[:, k, :], g1sd[bs][:, k, :],
                                         start=(k == 0), stop=(k == 1))
                    nc.scalar.activation(g2s[:], p[:], AF.Relu, bias=gb2s[:, 0:1])
                    g2sd[bs] = g2s

                def stage_z(bs):
                    bsl = slice(bs * BS, (bs + 1) * BS)
                    zp8 = z2ps.tile([E, BS], F32, tag="zp8", name="zp8")
                    nc.tensor.matmul(zp8[:], gw3s[:, :E], g2sd[bs][:],
                                     start=True, stop=True)
                    E8 = gtmp.tile([E, BS], F32R, tag="E8", name="E8")
                    nc.scalar.activation(E8[:], zp8[:], AF.Exp, bias=gb3s[:])
                    # unnormalized gates to DRAM for the expert loop
                    nc.scalar.dma_start(gates_dram[:, bsl], E8[:])
                    # softmax denominator via ones-matmul -> partition 0
                    sp = z2ps.tile([1, BS], F32, tag="sp", name="sp")
                    nc.tensor.matmul(sp[:], ones8[:], E8[:], start=True, stop=True)
                    # bounce to SBUF so the PSUM bank frees immediately
                    # (the reciprocal queues behind earlier DVE work)
                    spb = gtmp.tile([1, BS], F32, tag="spb", name="spb")
                    nc.scalar.activation(spb[:], sp[:], AF.Copy)
                    R = pers.tile([1, BS], F32, tag=f"R{bs}", name="R")
                    nc.vector.reciprocal(R[:], spb[:])
                    rlist.append(R)

                stage_g1(0)
                stage_g1(1)
                stage_g2(0)
                stage_g1(2)
                stage_g2(1)
                stage_z(0)
                stage_g1(3)
                stage_g2(2)
                stage_z(1)
                stage_g2(3)
                stage_z(2)
                stage_z(3)

            # ---------------- expert phase ----------------
            outTr = outT.rearrange("(mo p) b -> p mo b", p=128)
            with tc.tile_pool(name="hp", bufs=2) as hp, \
                 tc.tile_pool(name="bp", bufs=3) as bp, \
                 tc.tile_pool(name="eps", bufs=3, space="PSUM") as eps, \
                 tc.tile_pool(name="ops", bufs=2, space="PSUM") as ops:
                for e in range(E):
                    if e == 0:
                        w1, w2, w3 = w1_0, w2_0, w3_0
                    else:
                        w1, w2, w3 = load_expert(e)
                    if e == 2:
                        # denominator broadcasts, queued here so they
                        # neither delay the early expert weight DMAs nor
                        # the final scale at the tail
                        for bs2 in range(NBS):
                            nc.gpsimd.partition_broadcast(rbc[:, bs2, :],
                                                          rlist[bs2][:])
                    for bs in range(NBS):
                        bsl = slice(bs * BS, (bs + 1) * BS)
                        ge = bp.tile([1, BS], F32R, tag="ge")
                        nc.sync.dma_start(ge[:], gates_dram[e:e + 1, bsl])
                        gbc = bp.tile([128, BS], F32, tag="gbc")
                        nc.gpsimd.partition_broadcast(gbc[:].bitcast(F32R), ge[:])

                        h1s = hp.tile([128, KH, BS], F32R, tag="h")
                        for m in range(KH):
                            p = eps.tile([128, BS], F32, tag="h1p")
                            for k in range(KD):
                                nc.tensor.matmul(p[:], w1[:, k, m * 128:(m + 1) * 128],
                                                 xtb[bs][:, k, :],
                                                 start=(k == 0), stop=(k == KD - 1))
                            nc.scalar.activation(h1s[:, m], p[:], AF.Relu,
                                                 bias=eb1s[:, e, m:m + 1])
                        h2s = hp.tile([128, KH, BS], F32R, tag="h")
                        for m in range(KH):
                            p = eps.tile([128, BS], F32, tag="h2p")
                            for k in range(KH):
                                nc.tensor.matmul(p[:], w2[:, k, m * 128:(m + 1) * 128],
                                                 h1s[:, k, :],
                                                 start=(k == 0), stop=(k == KH - 1))
                            t2 = bp.tile([128, BS], F32, tag="t2")
                            nc.scalar.activation(t2[:], p[:], AF.Relu,
                                                 bias=eb2s[:, e, m:m + 1])
                            nc.vector.tensor_mul(h2s[:, m], t2[:], gbc[:])
                        for mo in range(O // 128):
                            p = ops.tile([128, BS], F32, tag="op")
                            nmm = KH + (1 if with_eb3 else 0)
                            for k in range(KH):
                                nc.tensor.matmul(p[:], w3[:, k, mo * 128:(mo + 1) * 128],
                                                 h2s[:, k, :],
                                                 start=(k == 0), stop=(k == nmm - 1))
                            if with_eb3:
                                nc.tensor.matmul(p[:], eb3s[:, e * O + mo * 128:
                                                            e * O + (mo + 1) * 128],
                                                 ge[:], start=False, stop=True)
                            if e == 0:
                                nc.vector.tensor_copy(outacc[:, mo, bsl], p[:])
                            elif e < E - 1:
                                nc.vector.tensor_add(outacc[:, mo, bsl],
                                                     outacc[:, mo, bsl], p[:])
                            else:
                                # last expert: fold in the softmax
                                # denominator and stream the chunk out
                                nc.vector.scalar_tensor_tensor(
                                    outacc[:, mo, bsl], p[:], 1.0,
                                    outacc[:, mo, bsl],
                                    mybir.AluOpType.mult,
                                    mybir.AluOpType.add)
                                nc.vector.tensor_mul(outacc[:, mo, bsl],
                                                     outacc[:, mo, bsl],
                                                     rbc[:, bs, :])
                        if e == E - 1:
                            nc.gpsimd.dma_start(outTr[:, :, bsl],
                                                outacc[:, :, bsl])

    nc.compile()
    _CACHE[key] = nc
    return nc


def kernel(x, gw1, gb1, gw2, gb2, gw3, gb3, ew1, eb1, ew2, eb2, ew3, eb3):
    x = np.asarray(x, dtype=np.float32)
    # [D_in, D_out] -> [128, KD, D_out] partition-major (16KB descriptors)
    pm = lambda w, kd: np.ascontiguousarray(
        np.asarray(w, np.float32).reshape(kd, 128, -1).transpose(1, 0, 2))
    gw1 = pm(gw1, KD)
    gw2 = pm(gw2, G1 // 128)
    gw3 = np.ascontiguousarray(np.asarray(gw3, dtype=np.float32))
    ew1 = np.ascontiguousarray(np.asarray(ew1, np.float32)
                               .reshape(E, KD, 128, H).transpose(0, 2, 1, 3))
    ew2 = np.ascontiguousarray(np.asarray(ew2, np.float32)
                               .reshape(E, KH, 128, H).transpose(0, 2, 1, 3))
    ew3 = np.ascontiguousarray(np.asarray(ew3, np.float32)
                               .reshape(E, KH, 128, O).transpose(0, 2, 1, 3))
    gb1t = np.ascontiguousarray(np.asarray(gb1, np.float32).reshape(2, 128).T)
    gb2t = np.ascontiguousarray(np.asarray(gb2, np.float32).reshape(1, 128).T)
    gb3t = np.ascontiguousarray(np.asarray(gb3, np.float32).reshape(E, 1))
    eb1t = np.ascontiguousarray(
        np.asarray(eb1, np.float32).reshape(E, 4, 128).transpose(2, 0, 1))
    eb2t = np.ascontiguousarray(
        np.asarray(eb2, np.float32).reshape(E, 4, 128).transpose(2, 0, 1))
    eb3f = np.ascontiguousarray(np.asarray(eb3, np.float32).reshape(1, E * O))

    with_eb3 = bool(np.any(eb3f))
    nc = _build(with_eb3)

    shared = {
        "gw1": gw1, "gw2": gw2, "gw3": gw3,
        "gb1t": gb1t, "gb2t": gb2t, "gb3t": gb3t,
        "ones8d": np.ones((E, 1), np.float32),
        "ew1": ew1, "ew2": ew2, "ew3": ew3,
        "eb1t": eb1t, "eb2t": eb2t, "eb3f": eb3f,
    }
    in_maps = []
    for c in range(NCORES):
        # [BL, D] -> [NBS, 128, KD, BS]: xT4[bs, p, ko, b] = x[bs*BS+b, ko*128+p]
        xc = x[c * BL:(c + 1) * BL, :].reshape(NBS, BS, KD, 128)
        xTc = np.ascontiguousarray(xc.transpose(0, 3, 2, 1))
        in_maps.append({"xT4": xTc, **shared})

    trace = os.environ.get("MOE_TRACE", "0") == "1"
    res = bass_utils.run_bass_kernel_spmd(
        nc, in_maps, core_ids=list(range(NCORES)), trace=trace)
    if trace:
        _CACHE["last_exec_time_ns"] = res.exec_time_ns
        _CACHE["last_results"] = res

    out = np.empty((B, O), dtype=np.float32)
    for c in range(NCORES):
        out[c * BL:(c + 1) * BL, :] = res.results[c]["outT"].T
    return out
